# revision 1
# baseline (speedup 1.0000x reference)
"""BoundaryAttentionModule Trainium2 kernel — moment-expansion algorithm.

Shapes (hardcoded): b=4, c=256, h=w=64 (HW=4096), boundary 128x128,
mid=64, out_ch=256. 8 cores: core = (batch bi = core//2, key-half kh = core%2).

Math: keys K[:,k] = W2 @ relu(kw1f * t_k + beta) depend on the SCALAR
boundary value t_k, so within each linear region S of the 64-breakpoint
piecewise map, E^T[k,j] = t_k * A_S[j] + B_S[j] where
  A_S[j] = sum_{i in S} tau*kw1f_i G[i,j],  B_S[j] = sum_{i in S} beta_i G[i,j]
(G = (W2^T Q_w) @ u, tau = max|t| folded into A). Then exactly
  U[k,j] = exp(B_S[j]) * sum_n ((t_k/tau)^n) * (A_S[j])^n / n!
so with moments Mo[(n,S),c] = sum_{k in S} (t_k/tau)^n v[k,c]/s_k and
w[(n,S),j] = exp(B_S[j]) A_S[j]^n/n!, the attention output collapses to
  P[c,j] = sum_{(n,S)} Mo[(n,S),c] w[(n,S),j].
Softmax denominators s[k] = sum_n (t_k/tau)^n sigma[n,S(k)] with
sigma = row-sums of w (free via accum_out). Truncation N=3 is exact to
well below the bf16 noise floor (final rel err ~1.4e-5; |tau*A| <~ 0.7).
Regions padded/merged to RP=32, so (n,S) = 4*32 = 128 = one partition tile.

Device per core: G2 = M@u, A/B = CAB@G2 (duo across alternating halves),
3-step w-chain on DVE, vT = u_k^T@vw^T (fills PE under the chain),
s = PWT@sigma, PW*(1/s), moment matmul, P matmul. Host: regions,
(t/tau)^n powers, j-roll per key-half, final gamma*P+u.
"""

import numpy as np

B, C, HW = 4, 256, 4096
KH = HW // 2          # 2048 keys per core
NKT = KH // 128       # 16 key tiles
MID = 64
RP = 32               # padded region count
NC = 4                # Taylor orders 0..3
COLS = RP * NC        # 128 = one partition tile

TRACE = False
TRACE_CORES = None
LAST_RESULTS = None

_BUILT = None


def _build():
    import concourse.bass as bass
    import concourse.tile as tile
    from concourse import bacc, mybir

    f32 = mybir.dt.float32
    bf16 = mybir.dt.bfloat16
    AF = mybir.ActivationFunctionType
    AX = mybir.AxisListType
    ALU = mybir.AluOpType

    nc = bacc.Bacc(
        "TRN2",
        target_bir_lowering=False,
        debug=False,
        enable_asserts=False,
        num_devices=8,
    )

    u_in = nc.dram_tensor("u_in", [C, HW], bf16, kind="ExternalInput").ap()
    mt_in = nc.dram_tensor("mt_in", [C, 2 * MID], bf16, kind="ExternalInput").ap()
    # cols 0:128 = 4 replicas of tau*CA^T (A lands in all four 32-row
    # partition groups), cols 128:160 = CB^T; rows duplicated for duo use.
    cab_in = nc.dram_tensor("cab_in", [2 * MID, 5 * RP], bf16, kind="ExternalInput").ap()
    vwt_in = nc.dram_tensor("vwt_in", [C, C], bf16, kind="ExternalInput").ap()
    pw_in = nc.dram_tensor("pw_in", [128, NKT * COLS], bf16, kind="ExternalInput").ap()
    pwt_in = nc.dram_tensor("pwt_in", [COLS, KH], bf16, kind="ExternalInput").ap()
    p_out = nc.dram_tensor("p_out", [C, HW], bf16, kind="ExternalOutput").ap()

    NJC = 8
    JW = HW // NJC        # 512-wide j chunks

    with tile.TileContext(nc) as tc:
        with (
            tc.tile_pool(name="sb", bufs=1) as sb,
            tc.tile_pool(name="ost", bufs=3) as osp,
            tc.tile_pool(name="big", bufs=2, space="PSUM") as bigp,
            tc.tile_pool(name="ab", bufs=2, space="PSUM") as abp,
            tc.tile_pool(name="pin", bufs=1, space="PSUM") as pinp,
        ):
            # ---- input DMAs ----
            mt = sb.tile([128, 2 * MID], bf16, tag="mt", name="mt")
            nc.sync.dma_start(mt[:], mt_in[0:128, :])
            mt1 = sb.tile([128, 2 * MID], bf16, tag="mt1", name="mt1")
            nc.sync.dma_start(mt1[:], mt_in[128:256, :])
            cab = sb.tile([2 * MID, 5 * RP], bf16, tag="cab", name="cab")
            nc.sync.dma_start(cab[:], cab_in[:, :])
            u0 = sb.tile([128, HW], bf16, tag="u0", name="u0")
            u1 = sb.tile([128, HW], bf16, tag="u1", name="u1")
            # progressive chunk sizes: first data lands ASAP to start G2
            for jo, w in ((0, 512), (512, 512), (1024, 1024), (2048, 2048)):
                nc.sync.dma_start(u0[:, jo : jo + w], u_in[0:128, jo : jo + w])
                nc.gpsimd.dma_start(u1[:, jo : jo + w], u_in[128:256, jo : jo + w])
            vwt0 = sb.tile([128, C], bf16, tag="vwt0", name="vwt0")
            nc.gpsimd.dma_start(vwt0[:], vwt_in[0:128, :])
            vwt1 = sb.tile([128, C], bf16, tag="vwt1", name="vwt1")
            nc.gpsimd.dma_start(vwt1[:], vwt_in[128:256, :])
            pwsb = sb.tile([128, NKT * COLS], bf16, tag="pwsb", name="pwsb")
            nc.scalar.dma_start(pwsb[:], pw_in[:, :])
            pwt = sb.tile([COLS, KH], bf16, tag="pwt", name="pwt")
            nc.scalar.dma_start(pwt[:], pwt_in[:, :])

            # ---- SBUF working tiles ----
            G2 = sb.tile([128, HW], bf16, tag="G2", name="G2")
            # tau*A replicated into all four 32-row partition groups so the
            # chain's two SBUF inputs always share a base partition.
            AsclR = sb.tile([128, HW], bf16, tag="AsclR", name="AsclR")
            W0 = sb.tile([128, HW], bf16, tag="W0", name="W0")   # n=0..3
            sacc0 = sb.tile([128, NJC], f32, tag="sacc0", name="sacc0")
            nc.gpsimd.memset(sacc0[:], 0.0)
            sig0 = sb.tile([128, 1], f32, tag="sig0", name="sig0")
            sigb0 = sb.tile([128, 1], bf16, tag="sigb0", name="sigb0")
            rinv = sb.tile([128, NKT], f32, tag="rinv", name="rinv")
            vtb = sb.tile([128, NKT * C], bf16, tag="vtb", name="vtb")
            pws = sb.tile([128, NKT * COLS], bf16, tag="pws", name="pws")
            mo0 = sb.tile([128, C], bf16, tag="mo0", name="mo0")
            kwsrc = sb.tile([32, 8], bf16, tag="kwsrc", name="kwsrc")
            scr = sb.tile([128, 512], bf16, tag="scr", name="scr")
            nc.vector.memset(scr[:], 0.0)   # DVE is idle at t=0

            spin = pinp.tile([128, 512], f32, tag="spin", name="spin")
            s_ps = spin[:, 0:NKT]                  # s accumulators
            mo_ps0 = spin[:, 256 : 256 + C]        # Mo accumulation

            # ---- per 512 j-chunk: G2 matmul+copy, A/B matmuls, expB, AsclR ----
            def g2_chunk(jc):
                jo = jc * JW
                pg = bigp.tile([128, JW], f32, tag="big", name=f"pg{jc}")
                nc.tensor.matmul(
                    pg[:], mt[:, :], u0[:, jo : jo + JW], start=True, stop=False
                )
                nc.tensor.matmul(
                    pg[:], mt1[:, :], u1[:, jo : jo + JW], start=False, stop=True
                )
                nc.vector.tensor_copy(G2[:, jo : jo + JW], pg[:, 0:JW])

            def ab_chunk(jc):
                # alternate G2 partition halves per chunk -> duo concurrency
                jo = jc * JW
                hb = 64 * (jc % 2)
                pa = abp.tile([128, JW], f32, tag="paA", name=f"paA{jc}")
                nc.tensor.matmul(
                    pa[:], cab[hb : hb + 64, 0:128],
                    G2[hb : hb + 64, jo : jo + JW], start=True, stop=True,
                )
                pb = abp.tile([32, JW], f32, tag="paB", name=f"paB{jc}")
                nc.tensor.matmul(
                    pb[:], cab[hb : hb + 64, 128:160],
                    G2[hb : hb + 64, jo : jo + JW], start=True, stop=True,
                )
                nc.scalar.activation(
                    W0[0:32, jo : jo + JW], pb[0:32, 0:JW], AF.Exp,
                    accum_out=sacc0[0:32, jc : jc + 1],
                )
                # alternate the 4-replica copy between ACT and DVE per chunk
                if jc % 2 == 0:
                    nc.scalar.copy(AsclR[:, jo : jo + JW], pa[:, 0:JW])
                else:
                    nc.vector.tensor_copy(AsclR[:, jo : jo + JW], pa[:, 0:JW])

            def chain_step(n, half):
                jo, w = half * (HW // 2), HW // 2
                dst = W0[n * 32 : (n + 1) * 32, jo : jo + w]
                src = W0[(n - 1) * 32 : n * 32, jo : jo + w]
                rep = AsclR[(n - 1) * 32 : n * 32, jo : jo + w]
                nc.vector.scalar_tensor_tensor(
                    dst, src, 1.0 / n, rep,
                    op0=ALU.mult, op1=ALU.mult,
                    accum_out=sacc0[n * 32 : (n + 1) * 32, half : half + 1],
                )

            def keep_warm(n, half):
                # Tiny matmul data-dependent on chain step (n, half) via a
                # 1-col DVE copy to a base-0 tile: spaces PE activity through
                # the chain so HAM stays at K=8/8.
                i = 2 * (n - 1) + half
                nc.vector.tensor_copy(
                    kwsrc[:, i : i + 1],
                    W0[n * 32 : n * 32 + 32, half * (HW // 2) : half * (HW // 2) + 1],
                )
                pz = abp.tile([32, JW], f32, tag="paB", name=f"kw{i}")
                nc.tensor.matmul(
                    pz[0:1, 0:64], kwsrc[:, i : i + 1], u0[0:32, 0:64],
                    start=True, stop=True,
                )

            # ---- vT matmuls (independent of chain; fills PE) ----
            def vt_tile(kt):
                pv = abp.tile([128, JW], f32, tag="paA", name=f"pv{kt}")
                ko = kt * 128
                nc.tensor.matmul(
                    pv[:, 0:C], u0[:, ko : ko + 128], vwt0[:],
                    start=True, stop=False,
                )
                nc.tensor.matmul(
                    pv[:, 0:C], u1[:, ko : ko + 128], vwt1[:],
                    start=False, stop=True,
                )
                nc.scalar.copy(vtb[:, kt * C : (kt + 1) * C], pv[:, 0:C])

            # ---- HAM warm-up: scratch matmuls spanning the input-DMA window,
            # so the PE reaches (and holds) K=8/8 before real work lands ----
            for i in range(11):
                pwm = bigp.tile([128, JW], f32, tag="big", name=f"warm{i}")
                nc.tensor.matmul(
                    pwm[:], scr[:, 0:128], scr[:, 0:512], start=True, stop=True
                )

            # Emission order drives the Tile scheduler's priorities.
            # vT tiles (keys = u cols 0..KH-1; host rolls u's j axis per core
            # so its key half leads, and un-rolls P afterward) interleave into
            # the front so the PE FIFO always has ready work.
            for jc in range(NJC):
                g2_chunk(jc)
                ab_chunk(jc)
                vt_tile(2 * jc)
                vt_tile(2 * jc + 1)
            # chain on DVE in half-width steps; keep-warm matmuls every
            # ~2.2us bridge the PE through the chain.  After step (2,0)
            # (~60% through), a scratch burst re-warms the PE so the s/Mo/P
            # tail starts at K=8/8 — the FIFO position after keep_warm(2,0)
            # is itself the gate.
            for n in range(1, NC):
                for half in range(2):
                    chain_step(n, half)
                    keep_warm(n, half)
                    if n == 2 and half == 0:
                        for i in range(8):
                            pwm = bigp.tile([128, JW], f32, tag="big",
                                            name=f"tw{i}")
                            nc.tensor.matmul(
                                pwm[:], scr[:, 0:128], scr[:, 0:512],
                                start=True, stop=True,
                            )

            # ---- sigma -> s -> rinv ----
            nc.vector.reduce_sum(sig0[:], sacc0[:], axis=AX.X)
            nc.vector.tensor_copy(sigb0[:], sig0[:])
            for kt in range(NKT):
                nc.tensor.matmul(
                    s_ps[:, kt : kt + 1],
                    pwt[:, kt * 128 : (kt + 1) * 128], sigb0[:],
                    start=True, stop=True,
                )
            nc.vector.reciprocal(rinv[:], s_ps[:])

            # ---- pws scaling interleaved with the moment matmul ----
            for kt in range(NKT):
                nc.vector.tensor_scalar(
                    pws[:, kt * COLS : (kt + 1) * COLS],
                    pwsb[:, kt * COLS : (kt + 1) * COLS],
                    rinv[:, kt : kt + 1], None, op0=ALU.mult,
                )
                nc.tensor.matmul(
                    mo_ps0[:],
                    pws[:, kt * COLS : (kt + 1) * COLS],
                    vtb[:, kt * C : (kt + 1) * C],
                    start=(kt == 0), stop=(kt == NKT - 1),
                )
            nc.scalar.copy(mo0[:], mo_ps0[:])

            # ---- P = Mo^T @ W -> DRAM (1024-wide output groups) ----
            for ct in range(2):
                for jg in range(4):
                    jo = jg * 1024
                    ost = osp.tile([128, 1024], bf16, tag="ost", name=f"ost{ct}_{jg}")
                    for q in range(2):
                        sl = slice(q * 512, (q + 1) * 512)
                        js = jo + q * 512
                        pp = bigp.tile([128, JW], f32, tag="big",
                                       name=f"pp{ct}_{jg}_{q}")
                        nc.tensor.matmul(
                            pp[:],
                            mo0[:, ct * 128 : (ct + 1) * 128],
                            W0[:, js : js + 512],
                            start=True, stop=True,
                        )
                        if q == 0:
                            nc.scalar.copy(ost[:, sl], pp[:])
                        else:
                            nc.vector.tensor_copy(ost[:, sl], pp[:])
                    q_eng = nc.sync if jg % 2 == 0 else nc.gpsimd
                    q_eng.dma_start(
                        p_out[ct * 128 : (ct + 1) * 128, jo : jo + 1024],
                        ost[:, 0:1024],
                    )

    nc.compile()
    return nc


def _get_built():
    global _BUILT
    if _BUILT is None:
        _BUILT = _build()
    return _BUILT


def _regions(kw1f, beta, tmin, tmax):
    """Region edges (sorted breakpoints in range, capped at RP-1) and the
    per-region active-set midpoints."""
    bp = -beta / np.where(np.abs(kw1f) < 1e-30, 1e-30, kw1f)
    inr = np.sort(bp[(bp > tmin) & (bp < tmax)])
    while len(inr) > RP - 1:       # merge narrowest adjacent regions
        gaps = np.diff(np.concatenate([[tmin], inr, [tmax]]))
        i = int(np.argmin(gaps[:-1] + gaps[1:]))
        inr = np.delete(inr, i)
    full = np.concatenate([[tmin - 1.0], inr, [tmax + 1.0]])
    tmid = 0.5 * (full[:-1] + full[1:])
    return inr, tmid


def _host_prep(boundary_map, uncertainty_map, key_w1, bn_scale, bn_bias,
               bn_mean, bn_var, key_w2, query_w, value_w):
    import ml_dtypes

    bf = ml_dtypes.bfloat16
    b, c, h, w = uncertainty_map.shape
    H0 = boundary_map.shape[2]
    idx = (np.arange(h) * H0) // h
    bm = boundary_map[:, 0][:, idx][:, :, idx].reshape(b, h * w).astype(np.float64)

    inv = bn_scale.astype(np.float64) / np.sqrt(bn_var.astype(np.float64) + 1e-5)
    beta = bn_bias.astype(np.float64) - bn_mean.astype(np.float64) * inv
    kw1f = key_w1[:, 0].astype(np.float64) * inv
    m_t = np.ascontiguousarray((key_w2.T @ query_w).T).astype(np.float64)  # [256, 64]
    m_t2 = np.concatenate([m_t, m_t], axis=1).astype(bf)                   # [256, 128]
    vw_t = np.ascontiguousarray(value_w.T).astype(bf)                      # [256, 256]

    in_maps = []
    for core in range(8):
        bi, kh = core // 2, core % 2
        t_full = bm[bi]
        tau = np.abs(t_full).max()
        edges, tmid = _regions(kw1f, beta, t_full.min(), t_full.max())
        R = len(edges) + 1
        masks = (kw1f[None, :] * tmid[:, None] + beta[None, :]) > 0   # [R, 64]
        ca = (masks * kw1f[None, :]) * tau                            # [R, 64]
        cb = masks * beta[None, :]
        cabm = np.zeros((MID, 5 * RP), np.float64)
        for r in range(4):                    # 4 replicas of tau*CA^T
            cabm[:, r * RP : r * RP + R] = ca.T
        cabm[:, 4 * RP : 4 * RP + R] = cb.T
        cab2 = np.concatenate([cabm, cabm], axis=0).astype(bf)        # [128, 160]

        tk = t_full[kh * KH : (kh + 1) * KH]
        reg = np.searchsorted(edges, tk)                              # [2048]
        tp = np.empty((NC, KH), np.float64)
        tp[0] = 1.0
        for n in range(1, NC):
            tp[n] = tp[n - 1] * (tk / tau)
        pw = np.zeros((KH, COLS), np.float64)
        pw[np.arange(KH)[None, :].repeat(NC, 0).ravel(),
           (np.arange(NC)[:, None] * RP + reg[None, :]).ravel()] = tp.ravel()
        # device layout: [128, NKT*COLS] (k-tile t at cols t*COLS)
        pw_dev = pw.reshape(NKT, 128, COLS).transpose(1, 0, 2).reshape(128, NKT * COLS)

        u = uncertainty_map[bi].reshape(c, h * w)
        u = np.ascontiguousarray(np.roll(u, -kh * KH, axis=1)).astype(bf)
        in_maps.append({
            "u_in": u,
            "mt_in": m_t2,
            "cab_in": cab2,
            "vwt_in": vw_t,
            "pw_in": np.ascontiguousarray(pw_dev).astype(bf),
            "pwt_in": np.ascontiguousarray(pw.T).astype(bf),
        })
    return in_maps


def kernel(boundary_map, uncertainty_map, key_w1, bn_scale, bn_bias,
           bn_mean, bn_var, key_w2, query_w, value_w, gamma):
    global LAST_RESULTS
    from concourse.bass_utils import run_bass_kernel_spmd

    nc = _get_built()
    in_maps = _host_prep(
        np.asarray(boundary_map), np.asarray(uncertainty_map), np.asarray(key_w1),
        np.asarray(bn_scale), np.asarray(bn_bias), np.asarray(bn_mean),
        np.asarray(bn_var), np.asarray(key_w2), np.asarray(query_w),
        np.asarray(value_w),
    )
    kwargs = {}
    if TRACE:
        kwargs["trace"] = True
        if TRACE_CORES is not None:
            kwargs["trace_cores"] = TRACE_CORES
    res = run_bass_kernel_spmd(nc, in_maps, core_ids=list(range(8)), **kwargs)
    LAST_RESULTS = res

    b, c, h, w = uncertainty_map.shape
    g = np.float32(np.asarray(gamma).reshape(-1)[0])
    out = np.empty((b, c, h * w), np.float32)
    um = np.asarray(uncertainty_map)
    for bi in range(b):
        P = (res.results[2 * bi]["p_out"].astype(np.float32)
             + np.roll(res.results[2 * bi + 1]["p_out"].astype(np.float32),
                       KH, axis=1))
        out[bi] = g * P + um[bi].reshape(c, h * w)
    return out.reshape(b, c, h, w)



# revision 9
# speedup vs baseline: 1.3054x; 1.3054x over previous
"""BoundaryAttentionModule Trainium2 kernel — centered moment expansion, fp8 DR.

Shapes (hardcoded): b=4, c=256, h=w=64 (HW=4096), mid=64, out_ch=256.
8 cores: core = (batch bi = core//2, key-half kh = core%2); each core
handles its 2048 keys against all 4096 queries j.

Math: E^T[k,j] = t_k*A_S[j] + B_S[j] within ReLU-region S of the scalar
boundary value t_k.  Expansion is CENTERED per region: with region
center t_S and half-width h_S, U[k,j] = exp(B'_S[j]) * exp(d A'_S[j])
where B' = B + t_S A, A' = h_S A, d = (t_k - t_S)/h_S in [-1,1].  The
host splits wide regions (64 region slots) so |d A'| is tiny and TWO
Taylor orders suffice: U ~ W0 + d*W1, W0 = exp(B'), W1 = W0*A'.
Host folds M = key_w2^T @ query_w into CA/CB: A'/B' come straight from
u via one fp8 DoubleRow matmul each (contraction c=256), no G2.

W [128=(n,S), 4096]: rows 0:64 = W0 = exp(B') (ACT exp from psum, with
sigma0 via accum), rows 64:128 = W1 = W0*A' (two scalar_tensor_tensor
[64,2048] steps on DVE, sigma1 via accum).  s = pwt^T @ sigma via 16
1-col matmuls; pws = PSCALE*pw/s in fp8; Mo via fp8 DR pair matmuls
over keys; P = Mo^T @ W in bf16; output fp8 (host divides PSCALE).
"""

import numpy as np

B, C, HW = 4, 256, 4096
KH = HW // 2          # 2048 keys per core
NKT = KH // 128       # 16 key tiles
RP = 64               # region slots
NORD = 2              # Taylor orders 0..1 (centered)
BASIS = NORD * RP     # 128
PSCALE = 128.0        # pws scale folded out on host via gamma

TRACE = False
TRACE_CORES = None
LAST_RESULTS = None

_BUILT = None


def _build():
    import concourse.bass as bass
    import concourse.tile as tile
    from concourse import bacc, mybir

    f32 = mybir.dt.float32
    bf16 = mybir.dt.bfloat16
    f8 = mybir.dt.float8e4
    AF = mybir.ActivationFunctionType
    AX = mybir.AxisListType
    ALU = mybir.AluOpType
    DR = mybir.MatmulPerfMode.DoubleRow

    nc = bacc.Bacc(
        "TRN2",
        target_bir_lowering=False,
        debug=False,
        enable_asserts=False,
        num_devices=8,
    )

    u8_in = nc.dram_tensor("u8_in", [128, 2, HW], f8, kind="ExternalInput").ap()
    cab8_in = nc.dram_tensor("cab8_in", [128, 2, 2 * RP], f8, kind="ExternalInput").ap()
    vw8_in = nc.dram_tensor("vw8_in", [128, 2, C], f8, kind="ExternalInput").ap()
    pwsb8_in = nc.dram_tensor("pwsb8_in", [128, 2, KH // 2], f8, kind="ExternalInput").ap()
    pwt_in = nc.dram_tensor("pwt_in", [BASIS, KH], bf16, kind="ExternalInput").ap()
    p_out = nc.dram_tensor("p_out", [C, HW], f8, kind="ExternalOutput").ap()

    with tile.TileContext(nc) as tc:
        with (
            tc.tile_pool(name="sb", bufs=1) as sb,
            tc.tile_pool(name="ab", bufs=2, space="PSUM") as abp,
            tc.tile_pool(name="vt", bufs=2, space="PSUM") as vtp,
            tc.tile_pool(name="pin", bufs=1, space="PSUM") as pinp,
        ):
            # ---- SBUF tiles ----
            u8 = sb.tile([128, 2, HW], f8, tag="u8", name="u8")
            cab8 = sb.tile([128, 2, 2 * RP], f8, tag="cab8", name="cab8")
            vw8 = sb.tile([128, 2, C], f8, tag="vw8", name="vw8")
            pwsb8 = sb.tile([128, 2, KH // 2], f8, tag="pwsb8", name="pwsb8")
            pws8 = sb.tile([128, 2, KH // 2], f8, tag="pws8", name="pws8")
            pwt = sb.tile([BASIS, KH], bf16, tag="pwt", name="pwt")
            Af = sb.tile([64, HW], bf16, tag="Af", name="Af")
            W = sb.tile([128, HW], bf16, tag="W", name="W")
            vtb = sb.tile([128, 2, 8 * C], f8, tag="vtb", name="vtb")
            sacc = sb.tile([64, 12], f32, tag="sacc", name="sacc")
            sigf = sb.tile([64, 2], f32, tag="sigf", name="sigf")
            sigb = sb.tile([128, 1], bf16, tag="sigb", name="sigb")
            rinv = sb.tile([128, NKT], f32, tag="rinv", name="rinv")
            mo0 = sb.tile([128, C], bf16, tag="mo0", name="mo0")
            po = sb.tile([128, 2 * HW], f8, tag="po", name="po")
            scr = sb.tile([128, 512], bf16, tag="scr", name="scr")
            nc.vector.memset(scr[:], 0.0)

            spin = pinp.tile([128, 512], f32, tag="spin", name="spin")
            s_ps = spin[:, 0:NKT]
            mo_ps = spin[:, 256 : 256 + C]

            # ---- input DMAs ----
            # u chunk ci: j cols 512*ci; h-interleaved arrival 0,4,1,5,...
            nc.sync.dma_start(cab8[:], cab8_in[:, :, :])
            nc.gpsimd.dma_start(vw8[:], vw8_in[:, :, :])

            def uchunk(ci):
                j0 = 512 * ci
                return (u8[:, :, j0 : j0 + 512], u8_in[:, :, j0 : j0 + 512])

            for eng, ci in (
                (nc.sync, 0), (nc.gpsimd, 4),
                (nc.scalar, 1), (nc.scalar, 5),
                (nc.sync, 2), (nc.gpsimd, 6),
                (nc.sync, 3), (nc.gpsimd, 7),
            ):
                d, s = uchunk(ci)
                eng.dma_start(d, s)
            nc.scalar.dma_start(pwsb8[:], pwsb8_in[:, :, :])
            nc.scalar.dma_start(pwt[:], pwt_in[:, :])

            # ---- PE warm-up while inputs stream ----
            def warm(i, n=1):
                for k in range(n):
                    pwm = abp.tile([128, 512], f32, tag="pb", name=f"warm{i}_{k}")
                    nc.tensor.matmul(
                        pwm[:], scr[:, 0:128], scr[:, 0:512], start=True, stop=True
                    )

            warm("pre", 9)

            # ---- A/B matmuls (fp8 DR, contraction c=256) + exp + Af copy ----
            def ab_chunk(ci, k):
                j0 = 512 * ci
                atile = abp.tile([128, 512], f32, tag="pa", name=f"pa{ci}")
                btile = abp.tile([128, 512], f32, tag="pb", name=f"pb{ci}")
                nc.tensor.matmul(
                    atile[0:64, 0:512], cab8[:, :, 0:64],
                    u8[:, :, j0 : j0 + 512],
                    start=True, stop=True, perf_mode=DR,
                )
                nc.tensor.matmul(
                    btile[0:64, 0:512], cab8[:, :, 64:128],
                    u8[:, :, j0 : j0 + 512],
                    start=True, stop=True, perf_mode=DR,
                )
                nc.scalar.activation(
                    W[0:64, j0 : j0 + 512], btile[0:64, 0:512], AF.Exp,
                    accum_out=sacc[0:64, k : k + 1],
                )
                if k % 2 == 0:
                    nc.vector.tensor_copy(Af[0:64, j0 : j0 + 512], atile[0:64, 0:512])
                else:
                    nc.scalar.copy(Af[0:64, j0 : j0 + 512], atile[0:64, 0:512])

            def vt_tile(kt, dst_eng):
                pv = vtp.tile([128, C], f32, tag="pv", name=f"pv{kt}")
                nc.tensor.matmul(
                    pv[:, 0:C],
                    u8[:, :, kt * 128 : (kt + 1) * 128],
                    vw8[:, :, :],
                    start=True, stop=True, perf_mode=DR,
                )
                dst = vtb[:, kt % 2 : kt % 2 + 1, (kt // 2) * C : (kt // 2 + 1) * C]
                if dst_eng is nc.scalar:
                    dst_eng.copy(dst, pv[:, 0:C])
                else:
                    dst_eng.tensor_copy(dst, pv[:, 0:C])

            # chunk order: j-halves interleaved so keys (cols 0:2048) and
            # chain inputs both complete early
            CHUNKS = (0, 4, 1, 5, 2, 6, 3, 7)
            for k, ci in enumerate(CHUNKS):
                ab_chunk(ci, k)
                if ci < 4:
                    vt_tile(4 * ci + 0, nc.vector)
                    vt_tile(4 * ci + 1, nc.scalar)
                    vt_tile(4 * ci + 2, nc.vector)
                    vt_tile(4 * ci + 3, nc.scalar)
                warm(f"ab{k}", 1)

            # ---- chain: W1 = W0 * A' (two [64,2048] steps, sigma1 accum) ----
            for half in range(2):
                j0 = half * KH
                nc.vector.scalar_tensor_tensor(
                    W[64:128, j0 : j0 + KH], W[0:64, j0 : j0 + KH], 1.0,
                    Af[0:64, j0 : j0 + KH],
                    op0=ALU.mult, op1=ALU.mult,
                    accum_out=sacc[0:64, 8 + half : 9 + half],
                )
            warm("ch", 6)

            # ---- sigma -> sigb [128,1]: rows 0:64 n0, 64:128 n1 ----
            nc.vector.reduce_sum(sigf[0:64, 0:1], sacc[0:64, 0:8], axis=AX.X)
            nc.vector.tensor_tensor(
                sigf[0:64, 1:2], sacc[0:64, 8:9], sacc[0:64, 9:10], op=ALU.add
            )
            nc.vector.tensor_copy(sigb[0:64, 0:1], sigf[0:64, 0:1])
            nc.vector.tensor_copy(sigb[64:128, 0:1], sigf[0:64, 1:2])

            # ---- s = pwt^T @ sigma ; rinv ----
            for kt in range(NKT):
                nc.tensor.matmul(
                    s_ps[:, kt : kt + 1],
                    pwt[:, kt * 128 : (kt + 1) * 128], sigb[:],
                    start=True, stop=True,
                )
            warm("s", 2)
            nc.vector.reciprocal(rinv[:], s_ps[:])

            # ---- pws = pwsb * rinv (fp8), then moment (fp8 DR pairs) ----
            def pws_kt(kt):
                i, pt = kt % 2, kt // 2
                dst = pws8[:, i : i + 1, pt * 128 : (pt + 1) * 128]
                src = pwsb8[:, i : i + 1, pt * 128 : (pt + 1) * 128]
                r = rinv[:, kt : kt + 1]
                if kt % 4 == 3:
                    nc.scalar.activation(dst, src, AF.Copy, scale=r)
                elif kt % 4 == 1:
                    nc.gpsimd.tensor_scalar(dst, src, r, None, op0=ALU.mult)
                else:
                    nc.vector.tensor_scalar(dst, src, r, None, op0=ALU.mult)

            for kt in range(NKT):
                pws_kt(kt)
            for pt in range(8):
                nc.tensor.matmul(
                    mo_ps[:],
                    pws8[:, :, pt * 128 : (pt + 1) * 128],
                    vtb[:, :, pt * C : (pt + 1) * C],
                    start=(pt == 0), stop=(pt == 7), perf_mode=DR,
                )
            nc.scalar.copy(mo0[:], mo_ps[:])

            # ---- P = Mo^T @ W -> fp8 out ----
            for ct in range(2):
                for jg in range(8):
                    pp = abp.tile([128, 512], f32, tag="pa", name=f"pp{ct}_{jg}")
                    nc.tensor.matmul(
                        pp[:],
                        mo0[:, ct * 128 : (ct + 1) * 128],
                        W[:, jg * 512 : (jg + 1) * 512],
                        start=True, stop=True,
                    )
                    dst = po[:, ct * HW + jg * 512 : ct * HW + (jg + 1) * 512]
                    if jg % 2 == 0:
                        nc.scalar.copy(dst, pp[:])
                    else:
                        nc.vector.tensor_copy(dst, pp[:])
                    if jg % 2 == 1:
                        q_eng = nc.sync if jg % 4 == 1 else nc.gpsimd
                        q_eng.dma_start(
                            p_out[ct * 128 : (ct + 1) * 128,
                                  (jg - 1) * 512 : (jg + 1) * 512],
                            po[:, ct * HW + (jg - 1) * 512 : ct * HW + (jg + 1) * 512],
                        )

    nc.compile()
    return nc


def _get_built():
    global _BUILT
    if _BUILT is None:
        _BUILT = _build()
    return _BUILT


def _regions(kw1f, beta, t):
    """Region edges: ReLU breakpoints inside t-range, merged to <= RP-1,
    then wide regions split so max |t - center| shrinks (all slots used)."""
    tmin, tmax = t.min(), t.max()
    bp = -beta / np.where(np.abs(kw1f) < 1e-30, 1e-30, kw1f)
    inr = np.sort(bp[(bp > tmin) & (bp < tmax)])
    while len(inr) > RP - 1:
        gaps = np.diff(np.concatenate([[tmin], inr, [tmax]]))
        i = int(np.argmin(gaps[:-1] + gaps[1:]))
        inr = np.delete(inr, i)
    edges = list(inr)
    while len(edges) < RP - 1:
        full = np.concatenate([[tmin - 1e-9], edges, [tmax + 1e-9]])
        bi, bm, bsplit = -1, -1.0, None
        for i in range(len(full) - 1):
            selm = t[(t > full[i]) & (t <= full[i + 1])]
            if len(selm) < 2:
                continue
            c = 0.5 * (selm.min() + selm.max())
            m = np.abs(selm - c).max()
            if m > bm:
                bm, bi, bsplit = m, i, float(np.median(selm))
        if bi < 0:
            break
        edges.append(bsplit)
        edges.sort()
    return np.array(edges)


def _host_prep(boundary_map, uncertainty_map, key_w1, bn_scale, bn_bias,
               bn_mean, bn_var, key_w2, query_w, value_w):
    import ml_dtypes

    bf = ml_dtypes.bfloat16
    f8 = ml_dtypes.float8_e4m3
    b, c, h, w = uncertainty_map.shape
    H0 = boundary_map.shape[2]
    idx = (np.arange(h) * H0) // h
    bm = boundary_map[:, 0][:, idx][:, :, idx].reshape(b, h * w).astype(np.float64)

    inv = bn_scale.astype(np.float64) / np.sqrt(bn_var.astype(np.float64) + 1e-5)
    beta = bn_bias.astype(np.float64) - bn_mean.astype(np.float64) * inv
    kw1f = key_w1[:, 0].astype(np.float64) * inv
    m_t = key_w2.T.astype(np.float64) @ query_w.astype(np.float64)   # [64, 256]
    vw_t = np.ascontiguousarray(value_w.T.astype(np.float64))        # [256, 256]
    vw8 = vw_t.reshape(2, 128, C).transpose(1, 0, 2)                 # [128,2,256]

    in_maps = []
    for core in range(8):
        bi, kh = core // 2, core % 2
        t_full = bm[bi]
        tk = t_full[kh * KH : (kh + 1) * KH]
        u = uncertainty_map[bi].reshape(c, h * w).astype(np.float64)
        u = np.roll(u, -kh * KH, axis=1)
        u8 = u.reshape(2, 128, HW).transpose(1, 0, 2)                # [128,2,HW]

        edges = _regions(kw1f, beta, tk)
        R = len(edges) + 1
        reg = np.searchsorted(edges, tk)                             # [KH]
        lo = np.concatenate([[tk.min() - 1e-9], edges])
        hi = np.concatenate([edges, [tk.max() + 1e-9]])
        relu_mid = 0.5 * (lo + hi)
        masks = (kw1f[None, :] * relu_mid[:, None] + beta[None, :]) > 0  # [R,64]
        tc = np.zeros(R)
        hh = np.ones(R)
        for r_ in range(R):
            selk = tk[reg == r_]
            if len(selk):
                tc[r_] = 0.5 * (selk.min() + selk.max())
                hh[r_] = max(np.abs(selk - tc[r_]).max(), 1e-6)
        ca = masks * kw1f[None, :]                                   # [R,64]
        cb = masks * beta[None, :]
        camT = ((hh[:, None] * ca) @ m_t).T                          # [256, R]
        cbmT = ((cb + tc[:, None] * ca) @ m_t).T                     # [256, R]
        cabf = np.zeros((256, 2 * RP))
        cabf[:, 0:R] = camT
        cabf[:, RP : RP + R] = cbmT
        cab8 = cabf.reshape(2, 128, 2 * RP).transpose(1, 0, 2)       # [128,2,128]

        dlt = (tk - tc[reg]) / hh[reg]                               # [KH]
        pw = np.zeros((KH, BASIS))
        krange = np.arange(KH)
        pw[krange, reg] = 1.0
        pw[krange, RP + reg] = dlt
        # pwsb8 [128 kw, 2 pairmember, 8*128]: [kw, i, pt*128+bc] =
        #   PSCALE * pw[(2pt+i)*128+kw, bc]  (PSCALE keeps pws in fp8 range;
        #   host divides it back out via gamma)
        pwsb = (PSCALE * pw).reshape(8, 2, 128, BASIS).transpose(2, 1, 0, 3).reshape(
            128, 2, 8 * BASIS)
        in_maps.append({
            "u8_in": np.ascontiguousarray(u8).astype(f8),
            "cab8_in": np.ascontiguousarray(cab8).astype(f8),
            "vw8_in": np.ascontiguousarray(vw8).astype(f8),
            "pwsb8_in": np.ascontiguousarray(pwsb).astype(f8),
            "pwt_in": np.ascontiguousarray(pw.T).astype(bf),
        })
    return in_maps


def kernel(boundary_map, uncertainty_map, key_w1, bn_scale, bn_bias,
           bn_mean, bn_var, key_w2, query_w, value_w, gamma):
    global LAST_RESULTS
    from concourse.bass_utils import run_bass_kernel_spmd

    nc = _get_built()
    in_maps = _host_prep(
        np.asarray(boundary_map), np.asarray(uncertainty_map), np.asarray(key_w1),
        np.asarray(bn_scale), np.asarray(bn_bias), np.asarray(bn_mean),
        np.asarray(bn_var), np.asarray(key_w2), np.asarray(query_w),
        np.asarray(value_w),
    )
    kwargs = {}
    if TRACE:
        kwargs["trace"] = True
        if TRACE_CORES is not None:
            kwargs["trace_cores"] = TRACE_CORES
    res = run_bass_kernel_spmd(nc, in_maps, core_ids=list(range(8)), **kwargs)
    LAST_RESULTS = res

    b, c, h, w = uncertainty_map.shape
    g = np.float64(np.asarray(gamma).reshape(-1)[0]) / PSCALE
    out = np.empty((b, c, h * w), np.float32)
    um = np.asarray(uncertainty_map)
    for bi in range(b):
        P = (res.results[2 * bi]["p_out"].astype(np.float32)
             + np.roll(res.results[2 * bi + 1]["p_out"].astype(np.float32),
                       KH, axis=1))
        out[bi] = g * P + um[bi].reshape(c, h * w)
    return out.reshape(b, c, h, w)
